# revision 13
# baseline (speedup 1.0000x reference)
"""Causal Performer (FAVOR+) Trainium2 kernel.

Sharding: 8 cores = 2 (batch) x 4 (head groups of 4 heads).  Each core
computes its 4 heads for one batch and returns a partial [4096, 2048]
output (its heads' contribution through w_o) in bf16; the host sums
the 4 partials per batch in f32.

Structure: software-pipelined over 512-position sequence blocks.  Each
iteration issues SCAN(k-1) (sparse small matmuls) BEFORE PROJ(k)
(dense N=512 matmuls with no compute deps), so the scheduler fills
every scan stall with projection work.

  - Packed-head layout: the 4 heads' 8 random features live at
    partition offsets 32h of one [128, 512] tile.  Row 32h+8 of the
    feature tiles is exactly 1.0 (zero-padded weights -> exp(0)), which
    the Z-terms exploit with K=9 to add EPS to the denominator free.
  - Numerator computed in [seq, dk+1] orientation: lhsT = masked A^T
    chunks, rhs = krec-scaled vha with a ones column, so column 128 of
    each numerator chunk IS the denominator (no separate den matmuls,
    no cross-partition broadcast, per-partition scalar reciprocal).
  - k' normalizer (krec) is folded into vha (su uses raw kfs), so the
    A^T consumer is a plain cast except a triu multiply on the
    diagonal 128-block.
  - Each divided [s, dk] chunk is PE-transposed back to [dk, s] for
    the unchanged output projection.
  - Scan is interleaved per head (A^T+mask, then numerator chunks with
    inline divide/transpose) so every PSUM FIFO reuse is gated by a
    consumer that ran hundreds of ns ago; elementwise consumers are
    split across Scalar and Vector (GpSimd is useless here: ~1.7us
    dispatch overhead per op).
  - Host pre-rearranges x and weights to partition-major layouts so
    every DMA descriptor line is 4-16KB contiguous (sync-engine
    descriptor generation was a bottleneck with [D, S] layouts).
  - PSUM: qk-feat 1 bank + vproj 1 + scan-shared FIFO 3 + oproj 3.
  - Output partials in bf16; host sums in f32.
"""

import os
import numpy as np
import ml_dtypes

from concourse import bacc, mybir
import concourse.tile as tile
from concourse.bass import ts
from concourse.bass_utils import run_bass_kernel_spmd
from concourse.masks import make_identity

B, S, D = 2, 4096, 2048
H_PER = 4            # heads per core
DK = 128
NB = 8
SBLK = 512           # sequence block
NBLK = S // SBLK     # 8
NSUB = SBLK // 128   # 4 sub-chunks of 128
EPS = 1e-6

bf16 = mybir.dt.bfloat16
f32 = mybir.dt.float32

LAST_EXEC_TIME_NS = None
_CACHE = {}

KC = D // 128
KG = 4               # kc-chunks per x DMA group


def _build():
    nc = bacc.Bacc("TRN2", target_bir_lowering=False, debug=False)

    # partition-major host layouts: contiguous per-partition runs
    xq_d = nc.dram_tensor("xq", [NBLK, 128, KC, SBLK], bf16, kind="ExternalInput").ap()
    xk_d = nc.dram_tensor("xk", [NBLK, 128, KC, SBLK], bf16, kind="ExternalInput").ap()
    xv_d = nc.dram_tensor("xv", [NBLK, 128, KC, SBLK], bf16, kind="ExternalInput").ap()
    wqom_d = nc.dram_tensor("wqom", [128, KC, 128], bf16, kind="ExternalInput").ap()
    wkom_d = nc.dram_tensor("wkom", [128, KC, 128], bf16, kind="ExternalInput").ap()
    wv_d = nc.dram_tensor("wv", [128, KC, 512], bf16, kind="ExternalInput").ap()
    wo_d = nc.dram_tensor("wo", [128, H_PER, D], bf16, kind="ExternalInput").ap()
    mask_d = nc.dram_tensor("mask", [128, 128], f32, kind="ExternalInput").ap()
    part_d = nc.dram_tensor("part", [S, D], bf16, kind="ExternalOutput").ap()

    Exp = mybir.ActivationFunctionType.Exp

    with tile.TileContext(nc) as tc:
        with tc.tile_pool(name="const", bufs=1) as const, \
             tc.tile_pool(name="wpool", bufs=1) as wpool, \
             tc.tile_pool(name="state", bufs=1) as state, \
             tc.tile_pool(name="xpool", bufs=2) as xpool, \
             tc.tile_pool(name="vpool", bufs=2) as vpool, \
             tc.tile_pool(name="featpool", bufs=2) as featpool, \
             tc.tile_pool(name="atmpool", bufs=1) as atmpool, \
             tc.tile_pool(name="otpool", bufs=6) as otpool, \
             tc.tile_pool(name="osbpool", bufs=2) as osbpool, \
             tc.tile_pool(name="miscpool", bufs=2) as miscpool, \
             tc.tile_pool(name="scrpool", bufs=1) as scrpool, \
             tc.tile_pool(name="psqk", bufs=1, space="PSUM") as psqk, \
             tc.tile_pool(name="psv", bufs=1, space="PSUM") as psv, \
             tc.tile_pool(name="pscan", bufs=3, space="PSUM") as pscan, \
             tc.tile_pool(name="psop", bufs=3, space="PSUM") as psop:

            ident = const.tile([128, 128], bf16, name="ident")
            make_identity(nc, ident)
            mask_sb = const.tile([128, 128], f32, name="mask_sb")
            nc.sync.dma_start(mask_sb[:], mask_d[:])

            wqom_sb = wpool.tile([128, KC, 128], bf16, name="wqom_sb")
            nc.sync.dma_start(wqom_sb[:], wqom_d[:])
            wkom_sb = wpool.tile([128, KC, 128], bf16, name="wkom_sb")
            nc.sync.dma_start(wkom_sb[:], wkom_d[:])
            # wv/wo DMAs issued inside proj(0) (cold-start order)
            wv_sb = wpool.tile([128, KC, 512], bf16, name="wv_sb")
            wo_sb = wpool.tile([128, H_PER, D], bf16, name="wo_sb")

            # persistent scan state, head h at partitions 32h..32h+8:
            # cols 0:128 = Z, col 128 = z.  Row 32h+8 col 128 = EPS,
            # contracted against the all-ones feature row 32h+8 by the
            # K=9 Z-term matmuls -> den + EPS for free.
            Zsb = state.tile([128, 132], f32, name="Zsb")
            nc.vector.memset(Zsb[:], 0.0)
            Zb16 = state.tile([128, 132], bf16, name="Zb16")
            nc.vector.memset(Zb16[:], 0.0)
            # col 128 = EPS everywhere: rows 32h+8 keep it forever (the
            # K=9 Z-term EPS), rows 32h..32h+8 are overwritten with the
            # true z state at the first block's update (transient 1e-6
            # perturbation of block 0's denominator — negligible)
            nc.vector.memset(Zb16[:, 128:129], EPS)

            def proj(blk):
                """Projections for block blk: returns tiles dict."""
                xq_sb = xpool.tile([128, KC, SBLK], bf16, name=f"xq{blk}", tag="xq")
                xk_sb = xpool.tile([128, KC, SBLK], bf16, name=f"xk{blk}", tag="xk")
                xv_sb = xpool.tile([128, KC, SBLK], bf16, name=f"xv{blk}", tag="xv")
                # split x DMAs into kc-groups so first matmuls start early;
                # wv before xv (vproj needs both, wv is smaller)
                kg = 2 if blk == 0 else KG
                order = [(xq_sb, xq_d), (xk_sb, xk_d)]
                for xsb, xd in order:
                    for g in range(KC // kg):
                        nc.sync.dma_start(
                            xsb[:, g * kg:(g + 1) * kg, :],
                            xd[blk, :, g * kg:(g + 1) * kg, :])
                if blk == 0:
                    nc.sync.dma_start(wv_sb[:], wv_d[:])
                for g in range(KC // kg):
                    nc.sync.dma_start(
                        xv_sb[:, g * kg:(g + 1) * kg, :],
                        xv_d[blk, :, g * kg:(g + 1) * kg, :])
                if blk == 0:
                    nc.sync.dma_start(wo_sb[:], wo_d[:])

                # fused q/k feature projections: head h rows at 32h
                qf_p = psqk.tile([128, SBLK], f32, name=f"qfp{blk}", tag="qk")
                kf_p = psqk.tile([128, SBLK], f32, name=f"kfp{blk}", tag="qk")
                for dst, wsb, xsb in ((qf_p, wqom_sb, xq_sb), (kf_p, wkom_sb, xk_sb)):
                    for kc in range(KC):
                        nc.tensor.matmul(dst[:], wsb[:, kc, :], xsb[:, kc, :],
                                         start=(kc == 0), stop=(kc == KC - 1))
                qsq = scrpool.tile([128, SBLK], f32, name=f"qsq{blk}", tag="qsq")
                nc.scalar.square(qsq[:], qf_p[:])
                ksq = scrpool.tile([128, SBLK], f32, name=f"ksq{blk}", tag="ksq")
                nc.scalar.square(ksq[:], kf_p[:])
                qfe = featpool.tile([128, SBLK], bf16, name=f"qfe{blk}", tag="qfe")
                nc.scalar.activation(qfe[:], qsq[:], Exp, scale=-0.5)
                kfe = featpool.tile([128, SBLK], bf16, name=f"kfe{blk}", tag="kfe")
                nc.scalar.activation(kfe[:], ksq[:], Exp, scale=-0.5)

                # v projection: vha [s_sub(128), j, head, 132] (+ones col)
                vha = vpool.tile([128, NSUB, H_PER, 132], bf16, name=f"vha{blk}", tag="vha")
                for j in range(NSUB):
                    pp = psv.tile([128, SBLK], f32, name=f"pv{blk}_{j}", tag="pp")
                    for kc in range(KC):
                        nc.tensor.matmul(pp[:], xv_sb[:, kc, ts(j, 128)],
                                         wv_sb[:, kc, :],
                                         start=(kc == 0), stop=(kc == KC - 1))
                    nc.scalar.copy(vha[:, j, :, 0:128],
                                   pp.rearrange("p (h d) -> p h d", d=128))
                    nc.vector.memset(vha[:, j, :, 128:129], 1.0)

                return dict(qfe=qfe, kfe=kfe, vha=vha)

            def scan_oproj(blk, t):
                """Scan + output projection for block blk using tiles t."""
                s0 = blk * SBLK
                qfe, kfe, vha = t["qfe"], t["kfe"], t["vha"]

                # k features transposed to [s, f-packed] (feeds ksum + su)
                kfs = featpool.tile([128, NSUB, 128], bf16, name=f"kfs{blk}", tag="kfs")
                for j in range(NSUB):
                    kT_p = pscan.tile([128, 128], bf16, name=f"kT{blk}_{j}", tag="sc")
                    nc.tensor.transpose(kT_p[:], kfe[:, ts(j, 128)], ident[:])
                    nc.vector.tensor_copy(kfs[:, j, :], kT_p[:])

                # normalizers: ksum[s, j*4+h] over the 8 real features
                ksum = miscpool.tile([128, NSUB * H_PER], f32, name=f"ksum{blk}", tag="ksum")
                nc.vector.reduce_sum(
                    ksum[:].rearrange("p (a b) -> p a b", b=H_PER),
                    kfs[:].rearrange("p a (b c) -> p a b c", c=32)[:, :, :, 0:NB],
                    axis=mybir.AxisListType.X)
                nc.vector.tensor_scalar_add(ksum[:], ksum[:], EPS)
                krec = miscpool.tile([128, NSUB * H_PER], f32, name=f"krec{blk}", tag="krec")
                nc.vector.reciprocal(krec[:], ksum[:])

                # fold k-normalizer into vha (incl. ones col -> krec), so
                # A^T stays raw and su uses raw kfs
                for j in range(NSUB):
                    for h in range(H_PER):
                        nc.vector.tensor_scalar(
                            out=vha[:, j, h, 0:129],
                            in0=vha[:, j, h, 0:129],
                            scalar1=krec[:, 4 * j + h:4 * j + h + 1], scalar2=None,
                            op0=mybir.AluOpType.mult)

                # per-head: A^T chunks (diag triu-mult on Vector, casts on
                # Scalar), then numerator chunks in [s, dk+1] with the
                # denominator in col 128 (K=9 Z-term adds EPS via the
                # all-ones feature row), divided and PE-transposed inline
                # A^T i2-outer: consecutive heads hit distinct PE row
                # groups so the next LDWEIGHTS pulls ahead of the running
                # matmul (same-row-group order serializes LDW+stream)
                atm = {}
                for i2 in range(NSUB):
                    n_i = SBLK - 128 * i2
                    for h in range(H_PER):
                        at_p = pscan.tile([128, n_i], f32, name=f"at{blk}_{i2}_{h}", tag="sc")
                        nc.tensor.matmul(at_p[:], kfe[32 * h:32 * h + NB, ts(i2, 128)],
                                         qfe[32 * h:32 * h + NB, 128 * i2:SBLK],
                                         start=True, stop=True,
                                         tile_position=(32 * h, 0))
                        am = atmpool.tile([128, n_i], bf16, name=f"am{blk}_{i2}_{h}",
                                          tag=f"atm{i2}", bufs=4)
                        nc.vector.tensor_mul(am[:, 0:128], at_p[:, 0:128], mask_sb[:])
                        if n_i > 128:
                            nc.scalar.copy(am[:, 128:n_i], at_p[:, 128:n_i])
                        atm[(i2, h)] = am

                outT = []
                for h in range(H_PER):
                    oT = otpool.tile([128, SBLK], bf16, name=f"oT{blk}_{h}", tag="outT")
                    for j in range(NSUB):
                        nt = pscan.tile([128, 129], f32, name=f"nt{blk}_{h}_{j}", tag="sc")
                        for i2 in range(j + 1):
                            c0 = 128 * (j - i2)
                            nc.tensor.matmul(nt[:], atm[(i2, h)][:, c0:c0 + 128],
                                             vha[:, i2, h, 0:129],
                                             start=(i2 == 0), stop=False,
                                             skip_group_check=True)
                        nc.tensor.matmul(nt[:], qfe[32 * h:32 * h + NB + 1, ts(j, 128)],
                                         Zb16[32 * h:32 * h + NB + 1, 0:129],
                                         start=False, stop=True,
                                         tile_position=(32 * h, 0),
                                         skip_group_check=True)
                        rcol = miscpool.tile([128, 1], f32, name=f"rc{blk}_{h}_{j}",
                                             tag="rc", bufs=4)
                        nc.vector.reciprocal(rcol[:], nt[:, 128:129])
                        ot = otpool.tile([128, 128], bf16, name=f"ot{blk}_{h}_{j}",
                                         tag="ot", bufs=3)
                        nc.vector.tensor_scalar(
                            out=ot[:], in0=nt[:, 0:128],
                            scalar1=rcol[:], scalar2=None,
                            op0=mybir.AluOpType.mult)
                        oTp = pscan.tile([128, 128], bf16, name=f"oTp{blk}_{h}_{j}", tag="sc")
                        nc.tensor.transpose(oTp[:], ot[:], ident[:])
                        nc.scalar.copy(oT[:, ts(j, 128)], oTp[:])
                    outT.append(oT)

                # state update: all 4 heads into one bank (col-group packed)
                su_p = psop.tile([128, SBLK], f32, name=f"su{blk}", tag="op")
                nc.vector.memset(su_p[:], 0.0)
                for i2 in range(NSUB):
                    for h in range(H_PER):
                        nc.tensor.matmul(su_p[32 * h:32 * h + NB, 0:129],
                                         kfs[:, i2, 32 * h:32 * h + NB],
                                         vha[:, i2, h, 0:129],
                                         start=False,
                                         stop=(i2 == NSUB - 1 and h == H_PER - 1),
                                         tile_position=(0, 32 * h),
                                         skip_group_check=True)
                for h in range(H_PER):
                    nc.vector.tensor_add(Zsb[32 * h:32 * h + NB, 0:129],
                                         Zsb[32 * h:32 * h + NB, 0:129],
                                         su_p[32 * h:32 * h + NB, 0:129])
                for h in range(H_PER):
                    nc.scalar.copy(Zb16[32 * h:32 * h + NB, 0:129],
                                          Zsb[32 * h:32 * h + NB, 0:129])

                # output projection; staging split Scalar/Vector, DMA out
                # per 512-col chunk
                for j in range(NSUB):
                    osb = osbpool.tile([128, D], bf16, name=f"osb{blk}_{j}", tag="osb")
                    r0 = s0 + 128 * j
                    for c in range(4):
                        op = psop.tile([128, 512], f32, name=f"op{blk}_{j}_{c}", tag="op")
                        for h in range(H_PER):
                            nc.tensor.matmul(op[:], outT[h][:, ts(j, 128)],
                                             wo_sb[:, h, ts(c, 512)],
                                             start=(h == 0), stop=(h == H_PER - 1))
                        if c % 2 == 0:
                            nc.scalar.copy(osb[:, ts(c, 512)], op[:])
                        else:
                            nc.vector.tensor_copy(osb[:, ts(c, 512)], op[:])
                        nc.sync.dma_start(part_d[r0:r0 + 128, ts(c, 512)],
                                          osb[:, ts(c, 512)])

            # software pipeline: scan(k-1) issued before proj(k)
            prev = None
            for k in range(NBLK + 1):
                if k >= 1:
                    scan_oproj(k - 1, prev)
                if k < NBLK:
                    prev = proj(k)

    nc.compile()
    return nc


def _pad_feat(w):
    """[4, 8, D] head-feature weights -> [128, KC, 128] partition-major
    with head h at output cols 32h."""
    out = np.zeros((128, D), np.float32)
    for h in range(H_PER):
        out[32 * h:32 * h + NB] = w[h]
    # [128 feat, D] -> lhsT layout [D, 128] -> [p, kc, 128]
    return np.ascontiguousarray(
        out.T.reshape(KC, 128, 128).transpose(1, 0, 2))


def _pmajor(w):
    """[D, M] weight -> partition-major [128, KC, M]."""
    return np.ascontiguousarray(w.reshape(KC, 128, -1).transpose(1, 0, 2))


def _xmajor(x):
    """[S, D] activations -> [NBLK, 128, KC, SBLK] partition-major."""
    # x.T is [D, S]; chunk rows into (KC, 128) and cols into (NBLK, SBLK)
    xt = x.T.reshape(KC, 128, NBLK, SBLK)
    return np.ascontiguousarray(xt.transpose(2, 1, 0, 3))


def _prep_inputs(q, k, v, w_q, w_k, w_v, w_o, omega):
    """Host-side sharding: returns in_maps for the 8 cores."""
    bf = ml_dtypes.bfloat16
    mask = np.triu(np.ones((128, 128), np.float32))

    xs = []
    for b in range(B):
        xs.append((_xmajor(q[b]).astype(bf),
                   _xmajor(k[b]).astype(bf),
                   _xmajor(v[b]).astype(bf)))

    wq_h = w_q.reshape(16, DK, D)
    wk_h = w_k.reshape(16, DK, D)
    wqom = np.einsum('nd,hde->hne', omega, wq_h)
    wkom = np.einsum('nd,hde->hne', omega, wk_h)

    in_maps = []
    for core in range(8):
        b, g = divmod(core, 4)
        sl = slice(512 * g, 512 * (g + 1))
        hsl = slice(4 * g, 4 * (g + 1))
        xq, xk, xv = xs[b]
        # wo: [512, D] -> [4, 128, D] -> [128, 4, D]
        wo = np.ascontiguousarray(
            w_o[:, sl].T.reshape(H_PER, 128, D).transpose(1, 0, 2))
        in_maps.append({
            "xq": xq, "xk": xk, "xv": xv,
            "wqom": _pad_feat(wqom[hsl]).astype(bf),
            "wkom": _pad_feat(wkom[hsl]).astype(bf),
            "wv": _pmajor(np.ascontiguousarray(w_v[sl, :].T)).astype(bf),
            "wo": wo.astype(bf),
            "mask": mask,
        })
    return in_maps


def kernel(q, k, v, w_q, w_k, w_v, w_o, omega):
    global LAST_EXEC_TIME_NS
    q, k, v = np.asarray(q), np.asarray(k), np.asarray(v)
    w_q, w_k, w_v, w_o = (np.asarray(a) for a in (w_q, w_k, w_v, w_o))
    omega = np.asarray(omega)

    if "nc" not in _CACHE:
        _CACHE["nc"] = _build()
    nc = _CACHE["nc"]

    in_maps = _prep_inputs(q, k, v, w_q, w_k, w_v, w_o, omega)
    trace = bool(os.environ.get("BASS_KERNEL_TRACE"))
    res = run_bass_kernel_spmd(nc, in_maps, core_ids=list(range(8)), trace=trace)
    LAST_EXEC_TIME_NS = res.exec_time_ns

    out = np.zeros((B, S, D), np.float32)
    for core in range(8):
        b = core // 4
        out[b] += res.results[core]["part"].astype(np.float32)
    return out


# revision 14
# speedup vs baseline: 1.1835x; 1.1835x over previous
"""Causal Performer (FAVOR+) Trainium2 kernel.

Sharding: 8 cores = 2 (batch) x 4 (head groups of 4 heads).  Each core
computes its 4 heads for one batch and returns a partial [4096, 2048]
output (its heads' contribution through w_o) in bf16; the host sums
the 4 partials per batch in f32.

Structure: software-pipelined over 512-position sequence blocks.  Each
iteration issues SCAN(k-1) (sparse small matmuls) BEFORE PROJ(k)
(dense N=512 matmuls with no compute deps), so the scheduler fills
every scan stall with projection work.

  - Packed-head layout: the 4 heads' 8 random features live at
    partition offsets 32h of one [128, 512] tile.  Row 32h+8 of the
    feature tiles is exactly 1.0 (zero-padded weights -> exp(0)), which
    the Z-terms exploit with K=9 to add EPS to the denominator free.
  - Numerator computed in [seq, dk+1] orientation: lhsT = masked A^T
    chunks, rhs = krec-scaled vha with a ones column, so column 128 of
    each numerator chunk IS the denominator (no separate den matmuls,
    no cross-partition broadcast, per-partition scalar reciprocal).
  - k' normalizer (krec) is folded into vha (su uses raw kfs), so the
    A^T consumer is a plain cast except a triu multiply on the
    diagonal 128-block.
  - Each divided [s, dk] chunk is PE-transposed back to [dk, s] for
    the unchanged output projection.
  - Scan is interleaved per head (A^T+mask, then numerator chunks with
    inline divide/transpose) so every PSUM FIFO reuse is gated by a
    consumer that ran hundreds of ns ago; elementwise consumers are
    split across Scalar and Vector (GpSimd is useless here: ~1.7us
    dispatch overhead per op).
  - Host pre-rearranges x and weights to partition-major layouts so
    every DMA descriptor line is 4-16KB contiguous (sync-engine
    descriptor generation was a bottleneck with [D, S] layouts).
  - PSUM: qk-feat 1 bank + vproj 1 + scan-shared FIFO 3 + oproj 3.
  - Output partials in bf16; host sums in f32.
"""

import os
import numpy as np
import ml_dtypes

from concourse import bacc, mybir
import concourse.tile as tile
from concourse.bass import ts
from concourse.bass_utils import run_bass_kernel_spmd
from concourse.masks import make_identity

B, S, D = 2, 4096, 2048
H_PER = 4            # heads per core
DK = 128
NB = 8
SBLK = 512           # sequence block
NBLK = S // SBLK     # 8
NSUB = SBLK // 128   # 4 sub-chunks of 128
EPS = 1e-6

bf16 = mybir.dt.bfloat16
f32 = mybir.dt.float32

LAST_EXEC_TIME_NS = None
_CACHE = {}

KC = D // 128
KG = 4               # kc-chunks per x DMA group


def _build():
    nc = bacc.Bacc("TRN2", target_bir_lowering=False, debug=False)

    # partition-major host layouts: contiguous per-partition runs
    xq_d = nc.dram_tensor("xq", [NBLK, 128, KC, SBLK], bf16, kind="ExternalInput").ap()
    xk_d = nc.dram_tensor("xk", [NBLK, 128, KC, SBLK], bf16, kind="ExternalInput").ap()
    xv_d = nc.dram_tensor("xv", [NBLK, 128, KC, SBLK], bf16, kind="ExternalInput").ap()
    wqom_d = nc.dram_tensor("wqom", [128, KC, 128], bf16, kind="ExternalInput").ap()
    wkom_d = nc.dram_tensor("wkom", [128, KC, 128], bf16, kind="ExternalInput").ap()
    wv_d = nc.dram_tensor("wv", [128, KC, 512], bf16, kind="ExternalInput").ap()
    wo_d = nc.dram_tensor("wo", [128, H_PER, D], bf16, kind="ExternalInput").ap()
    mask_d = nc.dram_tensor("mask", [128, 128], f32, kind="ExternalInput").ap()
    part_d = nc.dram_tensor("part", [S, D], bf16, kind="ExternalOutput").ap()

    Exp = mybir.ActivationFunctionType.Exp

    with tile.TileContext(nc) as tc:
        with tc.tile_pool(name="const", bufs=1) as const, \
             tc.tile_pool(name="wpool", bufs=1) as wpool, \
             tc.tile_pool(name="state", bufs=1) as state, \
             tc.tile_pool(name="xpool", bufs=2) as xpool, \
             tc.tile_pool(name="vpool", bufs=2) as vpool, \
             tc.tile_pool(name="featpool", bufs=2) as featpool, \
             tc.tile_pool(name="atmpool", bufs=1) as atmpool, \
             tc.tile_pool(name="otpool", bufs=6) as otpool, \
             tc.tile_pool(name="osbpool", bufs=2) as osbpool, \
             tc.tile_pool(name="miscpool", bufs=2) as miscpool, \
             tc.tile_pool(name="scrpool", bufs=1) as scrpool, \
             tc.tile_pool(name="psqk", bufs=1, space="PSUM") as psqk, \
             tc.tile_pool(name="psv", bufs=1, space="PSUM") as psv, \
             tc.tile_pool(name="pscan", bufs=3, space="PSUM") as pscan, \
             tc.tile_pool(name="psop", bufs=3, space="PSUM") as psop:

            ident = const.tile([128, 128], bf16, name="ident")
            make_identity(nc, ident)
            mask_sb = const.tile([128, 128], f32, name="mask_sb")
            nc.sync.dma_start(mask_sb[:], mask_d[:])

            wqom_sb = wpool.tile([128, KC, 128], bf16, name="wqom_sb")
            nc.sync.dma_start(wqom_sb[:], wqom_d[:])
            wkom_sb = wpool.tile([128, KC, 128], bf16, name="wkom_sb")
            nc.sync.dma_start(wkom_sb[:], wkom_d[:])
            # wv/wo DMAs issued inside proj(0) (cold-start order)
            wv_sb = wpool.tile([128, KC, 512], bf16, name="wv_sb")
            wo_sb = wpool.tile([128, H_PER, D], bf16, name="wo_sb")

            # persistent scan state, head h at partitions 32h..32h+8:
            # cols 0:128 = Z, col 128 = z.  Row 32h+8 col 128 = EPS,
            # contracted against the all-ones feature row 32h+8 by the
            # K=9 Z-term matmuls -> den + EPS for free.
            Zsb = state.tile([128, 132], f32, name="Zsb")
            nc.vector.memset(Zsb[:], 0.0)
            Zb16 = state.tile([128, 132], bf16, name="Zb16")
            nc.vector.memset(Zb16[:], 0.0)
            # col 128 = EPS everywhere: rows 32h+8 keep it forever (the
            # K=9 Z-term EPS), rows 32h..32h+8 are overwritten with the
            # true z state at the first block's update (transient 1e-6
            # perturbation of block 0's denominator — negligible)
            nc.vector.memset(Zb16[:, 128:129], EPS)

            def proj(blk):
                """Projections for block blk: returns tiles dict."""
                xq_sb = xpool.tile([128, KC, SBLK], bf16, name=f"xq{blk}", tag="xq")
                xk_sb = xpool.tile([128, KC, SBLK], bf16, name=f"xk{blk}", tag="xk")
                xv_sb = xpool.tile([128, KC, SBLK], bf16, name=f"xv{blk}", tag="xv")
                # split x DMAs into kc-groups so first matmuls start early;
                # wv before xv (vproj needs both, wv is smaller)
                kg = 2 if blk == 0 else KG
                order = [(xq_sb, xq_d), (xk_sb, xk_d)]
                for xsb, xd in order:
                    for g in range(KC // kg):
                        nc.sync.dma_start(
                            xsb[:, g * kg:(g + 1) * kg, :],
                            xd[blk, :, g * kg:(g + 1) * kg, :])
                if blk == 0:
                    nc.sync.dma_start(wv_sb[:], wv_d[:])
                for g in range(KC // kg):
                    nc.sync.dma_start(
                        xv_sb[:, g * kg:(g + 1) * kg, :],
                        xv_d[blk, :, g * kg:(g + 1) * kg, :])
                if blk == 0:
                    nc.sync.dma_start(wo_sb[:], wo_d[:])

                # fused q/k feature projections: head h rows at 32h
                qf_p = psqk.tile([128, SBLK], f32, name=f"qfp{blk}", tag="qk")
                kf_p = psqk.tile([128, SBLK], f32, name=f"kfp{blk}", tag="qk")
                for dst, wsb, xsb in ((qf_p, wqom_sb, xq_sb), (kf_p, wkom_sb, xk_sb)):
                    for kc in range(KC):
                        nc.tensor.matmul(dst[:], wsb[:, kc, :], xsb[:, kc, :],
                                         start=(kc == 0), stop=(kc == KC - 1))
                qsq = scrpool.tile([128, SBLK], f32, name=f"qsq{blk}", tag="qsq")
                nc.scalar.square(qsq[:], qf_p[:])
                ksq = scrpool.tile([128, SBLK], f32, name=f"ksq{blk}", tag="ksq")
                nc.scalar.square(ksq[:], kf_p[:])
                qfe = featpool.tile([128, SBLK], bf16, name=f"qfe{blk}", tag="qfe")
                nc.scalar.activation(qfe[:], qsq[:], Exp, scale=-0.5)
                kfe = featpool.tile([128, SBLK], bf16, name=f"kfe{blk}", tag="kfe")
                nc.scalar.activation(kfe[:], ksq[:], Exp, scale=-0.5)

                # v projection: vha [s_sub(128), j, head, 132] (+ones col)
                vha = vpool.tile([128, NSUB, H_PER, 132], bf16, name=f"vha{blk}", tag="vha")
                for j in range(NSUB):
                    pp = psv.tile([128, SBLK], f32, name=f"pv{blk}_{j}", tag="pp")
                    for kc in range(KC):
                        nc.tensor.matmul(pp[:], xv_sb[:, kc, ts(j, 128)],
                                         wv_sb[:, kc, :],
                                         start=(kc == 0), stop=(kc == KC - 1))
                    nc.scalar.copy(vha[:, j, :, 0:128],
                                   pp.rearrange("p (h d) -> p h d", d=128))
                    nc.vector.memset(vha[:, j, :, 128:129], 1.0)


                # k features transposed to [s, f-packed] (feeds ksum + su)
                kfs = featpool.tile([128, NSUB, 128], bf16, name=f"kfs{blk}", tag="kfs")
                for j in range(NSUB):
                    kT_p = pscan.tile([128, 128], bf16, name=f"kT{blk}_{j}", tag="sc")
                    nc.tensor.transpose(kT_p[:], kfe[:, ts(j, 128)], ident[:])
                    nc.vector.tensor_copy(kfs[:, j, :], kT_p[:])

                # normalizers: ksum[s, j*4+h] over the 8 real features
                ksum = miscpool.tile([128, NSUB * H_PER], f32, name=f"ksum{blk}", tag="ksum")
                nc.vector.reduce_sum(
                    ksum[:].rearrange("p (a b) -> p a b", b=H_PER),
                    kfs[:].rearrange("p a (b c) -> p a b c", c=32)[:, :, :, 0:NB],
                    axis=mybir.AxisListType.X)
                nc.vector.tensor_scalar_add(ksum[:], ksum[:], EPS)
                krec = miscpool.tile([128, NSUB * H_PER], f32, name=f"krec{blk}", tag="krec")
                nc.vector.reciprocal(krec[:], ksum[:])

                # fold k-normalizer into vha (incl. ones col -> krec), so
                # A^T stays raw and su uses raw kfs
                for j in range(NSUB):
                    for h in range(H_PER):
                        nc.vector.tensor_scalar(
                            out=vha[:, j, h, 0:129],
                            in0=vha[:, j, h, 0:129],
                            scalar1=krec[:, 4 * j + h:4 * j + h + 1], scalar2=None,
                            op0=mybir.AluOpType.mult)

                return dict(qfe=qfe, kfe=kfe, vha=vha, kfs=kfs)

            def scan_oproj(blk, t):
                """Scan + output projection for block blk using tiles t."""
                s0 = blk * SBLK
                qfe, kfe, vha, kfs = (t[x] for x in ("qfe", "kfe", "vha", "kfs"))

                # per-head: A^T chunks (diag triu-mult on Vector, casts on
                # Scalar), then numerator chunks in [s, dk+1] with the
                # denominator in col 128 (K=9 Z-term adds EPS via the
                # all-ones feature row), divided and PE-transposed inline
                outT = []
                for h in range(H_PER):
                    atm = {}
                    for i2 in range(NSUB):
                        n_i = SBLK - 128 * i2
                        at_p = pscan.tile([128, n_i], f32, name=f"at{blk}_{i2}_{h}", tag="sc")
                        nc.tensor.matmul(at_p[:], kfe[32 * h:32 * h + NB, ts(i2, 128)],
                                         qfe[32 * h:32 * h + NB, 128 * i2:SBLK],
                                         start=True, stop=True,
                                         tile_position=(32 * h, 0))
                        am = atmpool.tile([128, n_i], bf16, name=f"am{blk}_{i2}_{h}",
                                          tag=f"atm{i2}", bufs=4)
                        nc.vector.tensor_mul(am[:, 0:128], at_p[:, 0:128], mask_sb[:])
                        if n_i > 128:
                            nc.scalar.copy(am[:, 128:n_i], at_p[:, 128:n_i])
                        atm[i2] = am

                    oT = otpool.tile([128, SBLK], bf16, name=f"oT{blk}_{h}", tag="outT")
                    for j in range(NSUB):
                        nt = pscan.tile([128, 129], f32, name=f"nt{blk}_{h}_{j}", tag="sc")
                        for i2 in range(j + 1):
                            c0 = 128 * (j - i2)
                            nc.tensor.matmul(nt[:], atm[i2][:, c0:c0 + 128],
                                             vha[:, i2, h, 0:129],
                                             start=(i2 == 0), stop=False,
                                             skip_group_check=True)
                        nc.tensor.matmul(nt[:], qfe[32 * h:32 * h + NB + 1, ts(j, 128)],
                                         Zb16[32 * h:32 * h + NB + 1, 0:129],
                                         start=False, stop=True,
                                         tile_position=(32 * h, 0),
                                         skip_group_check=True)
                        rcol = miscpool.tile([128, 1], f32, name=f"rc{blk}_{h}_{j}",
                                             tag="rc", bufs=4)
                        nc.vector.reciprocal(rcol[:], nt[:, 128:129])
                        ot = otpool.tile([128, 128], bf16, name=f"ot{blk}_{h}_{j}",
                                         tag="ot", bufs=3)
                        nc.vector.tensor_scalar(
                            out=ot[:], in0=nt[:, 0:128],
                            scalar1=rcol[:], scalar2=None,
                            op0=mybir.AluOpType.mult)
                        oTp = pscan.tile([128, 128], bf16, name=f"oTp{blk}_{h}_{j}", tag="sc")
                        nc.tensor.transpose(oTp[:], ot[:], ident[:])
                        nc.scalar.copy(oT[:, ts(j, 128)], oTp[:])
                    outT.append(oT)

                # state update: all 4 heads into one bank (col-group packed)
                su_p = psop.tile([128, SBLK], f32, name=f"su{blk}", tag="op")
                nc.vector.memset(su_p[:], 0.0)
                for i2 in range(NSUB):
                    for h in range(H_PER):
                        nc.tensor.matmul(su_p[32 * h:32 * h + NB, 0:129],
                                         kfs[:, i2, 32 * h:32 * h + NB],
                                         vha[:, i2, h, 0:129],
                                         start=False,
                                         stop=(i2 == NSUB - 1 and h == H_PER - 1),
                                         tile_position=(0, 32 * h),
                                         skip_group_check=True)
                for h in range(H_PER):
                    nc.vector.tensor_add(Zsb[32 * h:32 * h + NB, 0:129],
                                         Zsb[32 * h:32 * h + NB, 0:129],
                                         su_p[32 * h:32 * h + NB, 0:129])
                for h in range(H_PER):
                    nc.scalar.copy(Zb16[32 * h:32 * h + NB, 0:129],
                                          Zsb[32 * h:32 * h + NB, 0:129])

                # output projection; staging split Scalar/Vector, DMA out
                # per 512-col chunk
                for j in range(NSUB):
                    osb = osbpool.tile([128, D], bf16, name=f"osb{blk}_{j}", tag="osb")
                    r0 = s0 + 128 * j
                    for c in range(4):
                        op = psop.tile([128, 512], f32, name=f"op{blk}_{j}_{c}", tag="op")
                        for h in range(H_PER):
                            nc.tensor.matmul(op[:], outT[h][:, ts(j, 128)],
                                             wo_sb[:, h, ts(c, 512)],
                                             start=(h == 0), stop=(h == H_PER - 1))
                        if c % 2 == 0:
                            nc.scalar.copy(osb[:, ts(c, 512)], op[:])
                        else:
                            nc.vector.tensor_copy(osb[:, ts(c, 512)], op[:])
                        nc.sync.dma_start(part_d[r0:r0 + 128, ts(c, 512)],
                                          osb[:, ts(c, 512)])

            # software pipeline: scan(k-1) issued before proj(k)
            prev = None
            for k in range(NBLK + 1):
                if k >= 1:
                    scan_oproj(k - 1, prev)
                if k < NBLK:
                    prev = proj(k)

    nc.compile()
    return nc


def _pad_feat(w):
    """[4, 8, D] head-feature weights -> [128, KC, 128] partition-major
    with head h at output cols 32h."""
    out = np.zeros((128, D), np.float32)
    for h in range(H_PER):
        out[32 * h:32 * h + NB] = w[h]
    # [128 feat, D] -> lhsT layout [D, 128] -> [p, kc, 128]
    return np.ascontiguousarray(
        out.T.reshape(KC, 128, 128).transpose(1, 0, 2))


def _pmajor(w):
    """[D, M] weight -> partition-major [128, KC, M]."""
    return np.ascontiguousarray(w.reshape(KC, 128, -1).transpose(1, 0, 2))


def _xmajor(x):
    """[S, D] activations -> [NBLK, 128, KC, SBLK] partition-major."""
    # x.T is [D, S]; chunk rows into (KC, 128) and cols into (NBLK, SBLK)
    xt = x.T.reshape(KC, 128, NBLK, SBLK)
    return np.ascontiguousarray(xt.transpose(2, 1, 0, 3))


def _prep_inputs(q, k, v, w_q, w_k, w_v, w_o, omega):
    """Host-side sharding: returns in_maps for the 8 cores."""
    bf = ml_dtypes.bfloat16
    mask = np.triu(np.ones((128, 128), np.float32))

    xs = []
    for b in range(B):
        xs.append((_xmajor(q[b]).astype(bf),
                   _xmajor(k[b]).astype(bf),
                   _xmajor(v[b]).astype(bf)))

    wq_h = w_q.reshape(16, DK, D)
    wk_h = w_k.reshape(16, DK, D)
    wqom = np.einsum('nd,hde->hne', omega, wq_h)
    wkom = np.einsum('nd,hde->hne', omega, wk_h)

    in_maps = []
    for core in range(8):
        b, g = divmod(core, 4)
        sl = slice(512 * g, 512 * (g + 1))
        hsl = slice(4 * g, 4 * (g + 1))
        xq, xk, xv = xs[b]
        # wo: [512, D] -> [4, 128, D] -> [128, 4, D]
        wo = np.ascontiguousarray(
            w_o[:, sl].T.reshape(H_PER, 128, D).transpose(1, 0, 2))
        in_maps.append({
            "xq": xq, "xk": xk, "xv": xv,
            "wqom": _pad_feat(wqom[hsl]).astype(bf),
            "wkom": _pad_feat(wkom[hsl]).astype(bf),
            "wv": _pmajor(np.ascontiguousarray(w_v[sl, :].T)).astype(bf),
            "wo": wo.astype(bf),
            "mask": mask,
        })
    return in_maps


def kernel(q, k, v, w_q, w_k, w_v, w_o, omega):
    global LAST_EXEC_TIME_NS
    q, k, v = np.asarray(q), np.asarray(k), np.asarray(v)
    w_q, w_k, w_v, w_o = (np.asarray(a) for a in (w_q, w_k, w_v, w_o))
    omega = np.asarray(omega)

    if "nc" not in _CACHE:
        _CACHE["nc"] = _build()
    nc = _CACHE["nc"]

    in_maps = _prep_inputs(q, k, v, w_q, w_k, w_v, w_o, omega)
    trace = bool(os.environ.get("BASS_KERNEL_TRACE"))
    res = run_bass_kernel_spmd(nc, in_maps, core_ids=list(range(8)), trace=trace)
    LAST_EXEC_TIME_NS = res.exec_time_ns

    out = np.zeros((B, S, D), np.float32)
    for core in range(8):
        b = core // 4
        out[b] += res.results[core]["part"].astype(np.float32)
    return out


# revision 15
# speedup vs baseline: 1.1929x; 1.0080x over previous
"""Causal Performer (FAVOR+) Trainium2 kernel.

Sharding: 8 cores = 2 (batch) x 4 (head groups of 4 heads).  Each core
computes its 4 heads for one batch and returns a partial [4096, 2048]
output (its heads' contribution through w_o) in bf16; the host sums
the 4 partials per batch in f32.

Structure: software-pipelined over 512-position sequence blocks.  Each
iteration issues SCAN(k-1) (sparse small matmuls) BEFORE PROJ(k)
(dense N=512 matmuls with no compute deps), so the scheduler fills
every scan stall with projection work.

  - Packed-head layout: the 4 heads' 8 random features live at
    partition offsets 32h of one [128, 512] tile.  Row 32h+8 of the
    feature tiles is exactly 1.0 (zero-padded weights -> exp(0)), which
    the Z-terms exploit with K=9 to add EPS to the denominator free.
  - Numerator computed in [seq, dk+1] orientation: lhsT = masked A^T
    chunks, rhs = krec-scaled vha with a ones column, so column 128 of
    each numerator chunk IS the denominator (no separate den matmuls,
    no cross-partition broadcast, per-partition scalar reciprocal).
  - k' normalizer (krec) is folded into vha (su uses raw kfs), so the
    A^T consumer is a plain cast except a triu multiply on the
    diagonal 128-block.
  - Each divided [s, dk] chunk is PE-transposed back to [dk, s] for
    the unchanged output projection.
  - Scan is interleaved per head (A^T+mask, then numerator chunks with
    inline divide/transpose) so every PSUM FIFO reuse is gated by a
    consumer that ran hundreds of ns ago; elementwise consumers are
    split across Scalar and Vector (GpSimd is useless here: ~1.7us
    dispatch overhead per op).
  - Host pre-rearranges x and weights to partition-major layouts so
    every DMA descriptor line is 4-16KB contiguous (sync-engine
    descriptor generation was a bottleneck with [D, S] layouts).
  - PSUM: qk-feat 1 bank + vproj 1 + scan-shared FIFO 3 + oproj 3.
  - Output partials in bf16; host sums in f32.
"""

import os
import numpy as np
import ml_dtypes

from concourse import bacc, mybir
import concourse.tile as tile
from concourse.bass import ts
from concourse.bass_utils import run_bass_kernel_spmd
from concourse.masks import make_identity

B, S, D = 2, 4096, 2048
H_PER = 4            # heads per core
DK = 128
NB = 8
SBLK = 512           # sequence block
NBLK = S // SBLK     # 8
NSUB = SBLK // 128   # 4 sub-chunks of 128
EPS = 1e-6

bf16 = mybir.dt.bfloat16
f32 = mybir.dt.float32

LAST_EXEC_TIME_NS = None
_CACHE = {}

KC = D // 128
KG = 4               # kc-chunks per x DMA group


def _build():
    nc = bacc.Bacc("TRN2", target_bir_lowering=False, debug=False)

    # partition-major host layouts: contiguous per-partition runs
    xq_d = nc.dram_tensor("xq", [NBLK, 128, KC, SBLK], bf16, kind="ExternalInput").ap()
    xk_d = nc.dram_tensor("xk", [NBLK, 128, KC, SBLK], bf16, kind="ExternalInput").ap()
    xv_d = nc.dram_tensor("xv", [NBLK, 128, KC, SBLK], bf16, kind="ExternalInput").ap()
    wqom_d = nc.dram_tensor("wqom", [128, KC, 128], bf16, kind="ExternalInput").ap()
    wkom_d = nc.dram_tensor("wkom", [128, KC, 128], bf16, kind="ExternalInput").ap()
    wv_d = nc.dram_tensor("wv", [128, KC, 512], bf16, kind="ExternalInput").ap()
    wo_d = nc.dram_tensor("wo", [128, H_PER, D], bf16, kind="ExternalInput").ap()
    mask_d = nc.dram_tensor("mask", [128, 128], f32, kind="ExternalInput").ap()
    part_d = nc.dram_tensor("part", [S, D], bf16, kind="ExternalOutput").ap()

    Exp = mybir.ActivationFunctionType.Exp

    with tile.TileContext(nc) as tc:
        with tc.tile_pool(name="const", bufs=1) as const, \
             tc.tile_pool(name="wpool", bufs=1) as wpool, \
             tc.tile_pool(name="state", bufs=1) as state, \
             tc.tile_pool(name="xpool", bufs=2) as xpool, \
             tc.tile_pool(name="vpool", bufs=2) as vpool, \
             tc.tile_pool(name="featpool", bufs=2) as featpool, \
             tc.tile_pool(name="atmpool", bufs=1) as atmpool, \
             tc.tile_pool(name="otpool", bufs=6) as otpool, \
             tc.tile_pool(name="osbpool", bufs=2) as osbpool, \
             tc.tile_pool(name="miscpool", bufs=2) as miscpool, \
             tc.tile_pool(name="scrpool", bufs=1) as scrpool, \
             tc.tile_pool(name="psqk", bufs=1, space="PSUM") as psqk, \
             tc.tile_pool(name="psv", bufs=1, space="PSUM") as psv, \
             tc.tile_pool(name="pscan", bufs=3, space="PSUM") as pscan, \
             tc.tile_pool(name="psop", bufs=3, space="PSUM") as psop:

            ident = const.tile([128, 128], bf16, name="ident")
            make_identity(nc, ident)
            mask_sb = const.tile([128, 128], f32, name="mask_sb")
            nc.sync.dma_start(mask_sb[:], mask_d[:])

            wqom_sb = wpool.tile([128, KC, 128], bf16, name="wqom_sb")
            nc.sync.dma_start(wqom_sb[:], wqom_d[:])
            wkom_sb = wpool.tile([128, KC, 128], bf16, name="wkom_sb")
            nc.sync.dma_start(wkom_sb[:], wkom_d[:])
            # wv/wo DMAs issued inside proj(0) (cold-start order)
            wv_sb = wpool.tile([128, KC, 512], bf16, name="wv_sb")
            wo_sb = wpool.tile([128, H_PER, D], bf16, name="wo_sb")

            # persistent scan state, head h at partitions 32h..32h+8:
            # cols 0:128 = Z, col 128 = z.  Row 32h+8 col 128 = EPS,
            # contracted against the all-ones feature row 32h+8 by the
            # K=9 Z-term matmuls -> den + EPS for free.
            Zsb = state.tile([128, 132], f32, name="Zsb")
            nc.vector.memset(Zsb[:], 0.0)
            Zb16 = state.tile([128, 132], bf16, name="Zb16")
            nc.vector.memset(Zb16[:], 0.0)
            # col 128 = EPS everywhere: rows 32h+8 keep it forever (the
            # K=9 Z-term EPS), rows 32h..32h+8 are overwritten with the
            # true z state at the first block's update (transient 1e-6
            # perturbation of block 0's denominator — negligible)
            nc.vector.memset(Zb16[:, 128:129], EPS)

            def proj(blk):
                """Projections for block blk: returns tiles dict."""
                xq_sb = xpool.tile([128, KC, SBLK], bf16, name=f"xq{blk}", tag="xq")
                xk_sb = xpool.tile([128, KC, SBLK], bf16, name=f"xk{blk}", tag="xk")
                xv_sb = xpool.tile([128, KC, SBLK], bf16, name=f"xv{blk}", tag="xv")
                # split x DMAs into kc-groups so first matmuls start early;
                # wv before xv (vproj needs both, wv is smaller)
                kg = 2 if blk == 0 else KG
                order = [(xq_sb, xq_d), (xk_sb, xk_d)]
                for xsb, xd in order:
                    for g in range(KC // kg):
                        nc.sync.dma_start(
                            xsb[:, g * kg:(g + 1) * kg, :],
                            xd[blk, :, g * kg:(g + 1) * kg, :])
                if blk == 0:
                    nc.sync.dma_start(wv_sb[:], wv_d[:])
                for g in range(KC // kg):
                    nc.sync.dma_start(
                        xv_sb[:, g * kg:(g + 1) * kg, :],
                        xv_d[blk, :, g * kg:(g + 1) * kg, :])
                if blk == 0:
                    nc.sync.dma_start(wo_sb[:], wo_d[:])

                # fused q/k feature projections: head h rows at 32h
                qf_p = psqk.tile([128, SBLK], f32, name=f"qfp{blk}", tag="qk")
                kf_p = psqk.tile([128, SBLK], f32, name=f"kfp{blk}", tag="qk")
                for dst, wsb, xsb in ((qf_p, wqom_sb, xq_sb), (kf_p, wkom_sb, xk_sb)):
                    for kc in range(KC):
                        nc.tensor.matmul(dst[:], wsb[:, kc, :], xsb[:, kc, :],
                                         start=(kc == 0), stop=(kc == KC - 1))
                qsq = scrpool.tile([128, SBLK], f32, name=f"qsq{blk}", tag="qsq")
                nc.scalar.square(qsq[:], qf_p[:])
                ksq = scrpool.tile([128, SBLK], f32, name=f"ksq{blk}", tag="ksq")
                nc.scalar.square(ksq[:], kf_p[:])
                qfe = featpool.tile([128, SBLK], bf16, name=f"qfe{blk}", tag="qfe")
                nc.scalar.activation(qfe[:], qsq[:], Exp, scale=-0.5)
                kfe = featpool.tile([128, SBLK], bf16, name=f"kfe{blk}", tag="kfe")
                nc.scalar.activation(kfe[:], ksq[:], Exp, scale=-0.5)

                # v projection: vha [s_sub(128), j, head, 132] (+ones col)
                vha = vpool.tile([128, NSUB, H_PER, 132], bf16, name=f"vha{blk}", tag="vha")
                for j in range(NSUB):
                    pp = psv.tile([128, SBLK], f32, name=f"pv{blk}_{j}", tag="pp")
                    for kc in range(KC):
                        nc.tensor.matmul(pp[:], xv_sb[:, kc, ts(j, 128)],
                                         wv_sb[:, kc, :],
                                         start=(kc == 0), stop=(kc == KC - 1))
                    nc.scalar.copy(vha[:, j, :, 0:128],
                                   pp.rearrange("p (h d) -> p h d", d=128))
                    nc.vector.memset(vha[:, j, :, 128:129], 1.0)


                # k features transposed to [s, f-packed] (feeds ksum + su)
                kfs = featpool.tile([128, NSUB, 128], bf16, name=f"kfs{blk}", tag="kfs")
                for j in range(NSUB):
                    kT_p = pscan.tile([128, 128], bf16, name=f"kT{blk}_{j}", tag="sc")
                    nc.tensor.transpose(kT_p[:], kfe[:, ts(j, 128)], ident[:])
                    nc.scalar.copy(kfs[:, j, :], kT_p[:])

                # normalizers: ksum[s, j*4+h] over the 8 real features
                ksum = miscpool.tile([128, NSUB * H_PER], f32, name=f"ksum{blk}", tag="ksum")
                nc.vector.reduce_sum(
                    ksum[:].rearrange("p (a b) -> p a b", b=H_PER),
                    kfs[:].rearrange("p a (b c) -> p a b c", c=32)[:, :, :, 0:NB],
                    axis=mybir.AxisListType.X)
                nc.vector.tensor_scalar_add(ksum[:], ksum[:], EPS)
                krec = miscpool.tile([128, NSUB * H_PER], f32, name=f"krec{blk}", tag="krec")
                nc.vector.reciprocal(krec[:], ksum[:])

                # fold k-normalizer into vha (incl. ones col -> krec), so
                # A^T stays raw and su uses raw kfs
                for j in range(NSUB):
                    for h in range(H_PER):
                        nc.vector.tensor_scalar(
                            out=vha[:, j, h, 0:129],
                            in0=vha[:, j, h, 0:129],
                            scalar1=krec[:, 4 * j + h:4 * j + h + 1], scalar2=None,
                            op0=mybir.AluOpType.mult)

                return dict(qfe=qfe, kfe=kfe, vha=vha, kfs=kfs)

            def scan_oproj(blk, t):
                """Scan + output projection for block blk using tiles t."""
                s0 = blk * SBLK
                qfe, kfe, vha, kfs = (t[x] for x in ("qfe", "kfe", "vha", "kfs"))

                # per-head: A^T chunks (diag triu-mult on Vector, casts on
                # Scalar), then numerator chunks in [s, dk+1] with the
                # denominator in col 128 (K=9 Z-term adds EPS via the
                # all-ones feature row), divided and PE-transposed inline
                outT = []
                ots = {}
                for h in range(H_PER):
                    atm = {}
                    for i2 in range(NSUB):
                        n_i = SBLK - 128 * i2
                        at_p = pscan.tile([128, n_i], f32, name=f"at{blk}_{i2}_{h}", tag="sc")
                        nc.tensor.matmul(at_p[:], kfe[32 * h:32 * h + NB, ts(i2, 128)],
                                         qfe[32 * h:32 * h + NB, 128 * i2:SBLK],
                                         start=True, stop=True,
                                         tile_position=(32 * h, 0))
                        am = atmpool.tile([128, n_i], bf16, name=f"am{blk}_{i2}_{h}",
                                          tag=f"atm{i2}", bufs=4)
                        nc.vector.tensor_mul(am[:, 0:128], at_p[:, 0:128], mask_sb[:])
                        if n_i > 128:
                            nc.scalar.copy(am[:, 128:n_i], at_p[:, 128:n_i])
                        atm[i2] = am

                    oT = otpool.tile([128, SBLK], bf16, name=f"oT{blk}_{h}", tag="outT")
                    for j in range(NSUB):
                        nt = pscan.tile([128, 129], f32, name=f"nt{blk}_{h}_{j}", tag="sc")
                        for i2 in range(j + 1):
                            c0 = 128 * (j - i2)
                            nc.tensor.matmul(nt[:], atm[i2][:, c0:c0 + 128],
                                             vha[:, i2, h, 0:129],
                                             start=(i2 == 0), stop=False,
                                             skip_group_check=True)
                        nc.tensor.matmul(nt[:], qfe[32 * h:32 * h + NB + 1, ts(j, 128)],
                                         Zb16[32 * h:32 * h + NB + 1, 0:129],
                                         start=False, stop=True,
                                         tile_position=(32 * h, 0),
                                         skip_group_check=True)
                        rcol = miscpool.tile([128, 1], f32, name=f"rc{blk}_{h}_{j}",
                                             tag="rc", bufs=4)
                        nc.vector.reciprocal(rcol[:], nt[:, 128:129])
                        # last block: no proj work follows to hide the
                        # recip/divide latency, so defer its transposes
                        # until every chain is in flight
                        last = blk == NBLK - 1
                        ot = otpool.tile([128, 128], bf16, name=f"ot{blk}_{h}_{j}",
                                         tag="otL" if last else "ot",
                                         bufs=16 if last else 3)
                        nc.vector.tensor_scalar(
                            out=ot[:], in0=nt[:, 0:128],
                            scalar1=rcol[:], scalar2=None,
                            op0=mybir.AluOpType.mult)
                        if last:
                            ots[(h, j)] = ot
                        else:
                            oTp = pscan.tile([128, 128], bf16, name=f"oTp{blk}_{h}_{j}", tag="sc")
                            nc.tensor.transpose(oTp[:], ot[:], ident[:])
                            nc.scalar.copy(oT[:, ts(j, 128)], oTp[:])
                    outT.append(oT)
                if blk == NBLK - 1:
                    for h in range(H_PER):
                        for j in range(NSUB):
                            oTp = pscan.tile([128, 128], bf16,
                                             name=f"oTpL_{h}_{j}", tag="sc")
                            nc.tensor.transpose(oTp[:], ots[(h, j)][:], ident[:])
                            nc.scalar.copy(outT[h][:, ts(j, 128)], oTp[:])

                # state update: all 4 heads into one bank (col-group packed)
                su_p = psop.tile([128, SBLK], f32, name=f"su{blk}", tag="op")
                nc.vector.memset(su_p[:], 0.0)
                for i2 in range(NSUB):
                    for h in range(H_PER):
                        nc.tensor.matmul(su_p[32 * h:32 * h + NB, 0:129],
                                         kfs[:, i2, 32 * h:32 * h + NB],
                                         vha[:, i2, h, 0:129],
                                         start=False,
                                         stop=(i2 == NSUB - 1 and h == H_PER - 1),
                                         tile_position=(0, 32 * h),
                                         skip_group_check=True)
                for h in range(H_PER):
                    nc.vector.tensor_add(Zsb[32 * h:32 * h + NB, 0:129],
                                         Zsb[32 * h:32 * h + NB, 0:129],
                                         su_p[32 * h:32 * h + NB, 0:129])
                for h in range(H_PER):
                    nc.scalar.copy(Zb16[32 * h:32 * h + NB, 0:129],
                                          Zsb[32 * h:32 * h + NB, 0:129])

                # output projection; staging split Scalar/Vector, DMA out
                # per 512-col chunk
                for j in range(NSUB):
                    osb = osbpool.tile([128, D], bf16, name=f"osb{blk}_{j}", tag="osb")
                    r0 = s0 + 128 * j
                    for c in range(4):
                        op = psop.tile([128, 512], f32, name=f"op{blk}_{j}_{c}", tag="op")
                        for h in range(H_PER):
                            nc.tensor.matmul(op[:], outT[h][:, ts(j, 128)],
                                             wo_sb[:, h, ts(c, 512)],
                                             start=(h == 0), stop=(h == H_PER - 1))
                        if c % 2 == 0:
                            nc.scalar.copy(osb[:, ts(c, 512)], op[:])
                        else:
                            nc.vector.tensor_copy(osb[:, ts(c, 512)], op[:])
                        nc.sync.dma_start(part_d[r0:r0 + 128, ts(c, 512)],
                                          osb[:, ts(c, 512)])

            # software pipeline: scan(k-1) issued before proj(k)
            prev = None
            for k in range(NBLK + 1):
                if k >= 1:
                    scan_oproj(k - 1, prev)
                if k < NBLK:
                    prev = proj(k)

    nc.compile()
    return nc


def _pad_feat(w):
    """[4, 8, D] head-feature weights -> [128, KC, 128] partition-major
    with head h at output cols 32h."""
    out = np.zeros((128, D), np.float32)
    for h in range(H_PER):
        out[32 * h:32 * h + NB] = w[h]
    # [128 feat, D] -> lhsT layout [D, 128] -> [p, kc, 128]
    return np.ascontiguousarray(
        out.T.reshape(KC, 128, 128).transpose(1, 0, 2))


def _pmajor(w):
    """[D, M] weight -> partition-major [128, KC, M]."""
    return np.ascontiguousarray(w.reshape(KC, 128, -1).transpose(1, 0, 2))


def _xmajor(x):
    """[S, D] activations -> [NBLK, 128, KC, SBLK] partition-major."""
    # x.T is [D, S]; chunk rows into (KC, 128) and cols into (NBLK, SBLK)
    xt = x.T.reshape(KC, 128, NBLK, SBLK)
    return np.ascontiguousarray(xt.transpose(2, 1, 0, 3))


def _prep_inputs(q, k, v, w_q, w_k, w_v, w_o, omega):
    """Host-side sharding: returns in_maps for the 8 cores."""
    bf = ml_dtypes.bfloat16
    mask = np.triu(np.ones((128, 128), np.float32))

    xs = []
    for b in range(B):
        xs.append((_xmajor(q[b]).astype(bf),
                   _xmajor(k[b]).astype(bf),
                   _xmajor(v[b]).astype(bf)))

    wq_h = w_q.reshape(16, DK, D)
    wk_h = w_k.reshape(16, DK, D)
    wqom = np.einsum('nd,hde->hne', omega, wq_h)
    wkom = np.einsum('nd,hde->hne', omega, wk_h)

    in_maps = []
    for core in range(8):
        b, g = divmod(core, 4)
        sl = slice(512 * g, 512 * (g + 1))
        hsl = slice(4 * g, 4 * (g + 1))
        xq, xk, xv = xs[b]
        # wo: [512, D] -> [4, 128, D] -> [128, 4, D]
        wo = np.ascontiguousarray(
            w_o[:, sl].T.reshape(H_PER, 128, D).transpose(1, 0, 2))
        in_maps.append({
            "xq": xq, "xk": xk, "xv": xv,
            "wqom": _pad_feat(wqom[hsl]).astype(bf),
            "wkom": _pad_feat(wkom[hsl]).astype(bf),
            "wv": _pmajor(np.ascontiguousarray(w_v[sl, :].T)).astype(bf),
            "wo": wo.astype(bf),
            "mask": mask,
        })
    return in_maps


def kernel(q, k, v, w_q, w_k, w_v, w_o, omega):
    global LAST_EXEC_TIME_NS
    q, k, v = np.asarray(q), np.asarray(k), np.asarray(v)
    w_q, w_k, w_v, w_o = (np.asarray(a) for a in (w_q, w_k, w_v, w_o))
    omega = np.asarray(omega)

    if "nc" not in _CACHE:
        _CACHE["nc"] = _build()
    nc = _CACHE["nc"]

    in_maps = _prep_inputs(q, k, v, w_q, w_k, w_v, w_o, omega)
    trace = bool(os.environ.get("BASS_KERNEL_TRACE"))
    res = run_bass_kernel_spmd(nc, in_maps, core_ids=list(range(8)), trace=trace)
    LAST_EXEC_TIME_NS = res.exec_time_ns

    out = np.zeros((B, S, D), np.float32)
    for core in range(8):
        b = core // 4
        out[b] += res.results[core]["part"].astype(np.float32)
    return out
